# revision 1
# baseline (speedup 1.0000x reference)
"""Causal self-attention Trainium2 kernel.

Problem: B=8, T=2048, C=512, H=8 heads (D=64), fp32.
  q = x@Wq.T ; k = x@Wk.T ; v = x@Wv.T  (per head)
  att = softmax(mask(q k^T / sqrt(D)))  ; y = att v ; out = y@Wp.T

Sharding: data-parallel over batch B across 8 NeuronCores (one batch
element per core, weights replicated). No collectives needed.

Per-core algorithm (everything stays on-chip; fp32r matmuls):
  - Host passes x[b].T ([C,T]) and the four W.T ([C_in,C_out]) so all
    matmuls contract over the partition dim without on-chip transposes.
  - qT/kT ([C,T]) and v ([T,C]) computed by projection matmuls.
  - Attention in "scores-transposed" layout: sT[k,q] = kT.T-block @ qT,
    exp via ScalarE (scale=1/sqrt(D) folded in, no max-subtraction --
    scores are O(1) here), causal handled by trimming whole block
    columns + one triangular mask multiply per diagonal-block pair.
  - The two heads of a pair run their QK matmuls on disjoint PE row
    groups (K=64 at base partitions 0/64) so consecutive matmuls
    overlap in the array.
  - P@V computed directly from the transposed-exp layout with a
    ones-augmented V, which also yields the softmax denominators.
  - Denominators, per head-pair (overlapped with the next pair):
    -> DRAM -> reciprocal -> DRAM -> partition-broadcast DMA -> row
    scale of yT; output projection from yT at the end.
"""

import numpy as np

import concourse.bass as bass
import concourse.bacc as bacc
import concourse.tile as tile
from concourse import mybir
from concourse.bass_utils import run_bass_kernel_spmd

B, T, C, H = 8, 2048, 512, 8
D = C // H          # 64
NT = T // 512       # 4 q-tiles of 512
NB = T // 128       # 16 k-blocks of 128
f32 = mybir.dt.float32
f32r = mybir.dt.float32r
EXP = mybir.ActivationFunctionType.Exp
N_CORES = 8


def build_nc():
    nc = bacc.Bacc(None)
    xT = nc.dram_tensor("xT", [C, T], f32r, kind="ExternalInput")
    wq = nc.dram_tensor("wqT", [C, C], f32r, kind="ExternalInput")
    wk = nc.dram_tensor("wkT", [C, C], f32r, kind="ExternalInput")
    wv = nc.dram_tensor("wvT", [C, C], f32r, kind="ExternalInput")
    wp = nc.dram_tensor("wpT", [C, C], f32r, kind="ExternalInput")
    out = nc.dram_tensor("out", [T, C], f32, kind="ExternalOutput")
    r_dram = nc.dram_tensor("r_dram", [H * NT, 512], f32)

    with tile.TileContext(nc) as tc:
        with tc.tile_pool(name="const", bufs=1) as constp, \
             tc.tile_pool(name="xw", bufs=1) as xw, \
             tc.tile_pool(name="vp", bufs=1) as vpool, \
             tc.tile_pool(name="kq", bufs=2) as kq, \
             tc.tile_pool(name="yp", bufs=1) as yp, \
             tc.tile_pool(name="expp", bufs=2) as expp, \
             tc.tile_pool(name="stg", bufs=3) as stg, \
             tc.tile_pool(name="bcp", bufs=4) as bcp, \
             tc.tile_pool(name="osb", bufs=3) as osb, \
             tc.tile_pool(name="lr", bufs=2) as lr, \
             tc.tile_pool(name="qkps", bufs=1, space="PSUM") as qkps, \
             tc.tile_pool(name="yps", bufs=1, space="PSUM") as yps, \
             tc.tile_pool(name="pps", bufs=2, space="PSUM") as pps:

            # ---- constants: [128, 256] = two copies of lower-tri keep mask
            tri = constp.tile([128, 256], f32, tag="tri")
            nc.gpsimd.memset(tri[:, :], 1.0)
            for half in range(2):
                sl = tri[:, half * 128:(half + 1) * 128]
                nc.gpsimd.affine_select(
                    out=sl, in_=sl, pattern=[[1, 128]], base=0,
                    channel_multiplier=-1,
                    compare_op=mybir.AluOpType.is_ge, fill=0.0)

            # ---- loads: small k/q weights first, then xT (so the first
            # projection matmul starts as soon as xT[0] lands), then v/p
            def load_w(dram, name):
                ws = []
                for ci in range(4):
                    t = xw.tile([128, C], f32r, tag=f"{name}{ci}")
                    nc.sync.dma_start(out=t[:, :], in_=dram[128 * ci:128 * (ci + 1), :])
                    ws.append(t)
                return ws

            xt = []
            for ci in range(4):
                t = xw.tile([128, T], f32r, tag=f"xT{ci}", name=f"xt{ci}")
                xt.append(t)

            def load_x_n(n):
                for ci in range(4):
                    nc.sync.dma_start(
                        out=xt[ci][:, 512 * n:512 * (n + 1)],
                        in_=xT[128 * ci:128 * (ci + 1), 512 * n:512 * (n + 1)])

            # interleave loads so the first k/q projection (needs wk/wq +
            # x columns 0:512 only) can start ~5us in, not after all of xT
            wkt = load_w(wk, "wk")
            load_x_n(0)
            wqt = load_w(wq, "wq")
            load_x_n(1)
            wvt = load_w(wv, "wv")
            load_x_n(2)
            load_x_n(3)
            wpt = load_w(wp, "wp")

            def kq_proj_n(dst_t, wt, p, n):
                ps = pps.tile([128, 512], f32, tag="proj", name="pproj")
                for ci in range(4):
                    nc.tensor.matmul(
                        ps[:, :],
                        wt[ci][:, 128 * p:128 * (p + 1)],
                        xt[ci][:, 512 * n:512 * (n + 1)],
                        start=(ci == 0), stop=(ci == 3))
                nc.vector.tensor_copy(dst_t[:, 512 * n:512 * (n + 1)], ps[:, :])

            def kq_proj(dst_t, wt, p):
                for n in range(NT):
                    kq_proj_n(dst_t, wt, p, n)

            # pair-0 k/q projection tiles (filled per-qn inside the loop)
            kts = {0: kq.tile([128, T], f32r, tag="k", name="kt")}
            qts = {0: kq.tile([128, T], f32r, tag="q", name="qt")}

            # ---- V projection: v_sb[tt] = [128, 8*65], head h at cols
            # [65h, 65h+64), ones column at 65h+64. Emitted in groups of 4
            # interleaved with pair-0 attention.
            vsb = [None] * NB

            def v_proj_group(qn):
                for tt in range(4 * qn, 4 * qn + 4):
                    ps = pps.tile([128, 512], f32, tag="proj", name="pproj")
                    for ci in range(4):
                        nc.tensor.matmul(ps[:, :],
                                         xt[ci][:, 128 * tt:128 * (tt + 1)],
                                         wvt[ci][:, :],
                                         start=(ci == 0), stop=(ci == 3))
                    vt = vpool.tile([128, 8 * (D + 1)], f32r,
                                    tag=f"v{tt}", name=f"v{tt}")
                    nc.vector.memset(vt[:, :].bitcast(f32), 1.0)
                    s3 = ps[:, :].rearrange("p (h d) -> p h d", h=H)
                    dst = vt[:, :].rearrange("p (h e) -> p h e", h=H)[:, :, 0:D]
                    nc.vector.tensor_copy(dst, s3)
                    vsb[tt] = vt

            yts = [yp.tile([128, T], f32r, tag=f"yT{i}", name=f"yT{i}")
                   for i in range(4)]

            def denom_pipeline(p, qn, lsq):
                """Reciprocal + broadcast + row-scale for (head pair, q-tile)."""
                r0 = 8 * p + 2 * qn
                rsq = lr.tile([16, 64], f32, tag="rsq", name="rsq")
                nc.vector.reciprocal(out=rsq[:, :], in_=lsq[:, :])
                nc.sync.dma_start(
                    out=r_dram[r0:r0 + 2, :].rearrange("r (a b) -> (r a) b", a=8),
                    in_=rsq[:, :])
                for h in (2 * p, 2 * p + 1):
                    o = D * (h % 2)
                    r = r0 + (h % 2)
                    bt = bcp.tile([128, 512], f32, tag="bc", name="bc")
                    nc.sync.dma_start(
                        out=bt[o:o + D, :],
                        in_=r_dram[r:r + 1, :].to_broadcast([D, 512]))
                    ysl = yts[p][o:o + D, 512 * qn:512 * (qn + 1)]
                    nc.vector.tensor_mul(ysl, ysl.bitcast(f32), bt[o:o + D, :])

            # ---- per head-pair attention (both heads interleaved so their
            # K=64 QK matmuls land on disjoint PE row groups back-to-back)
            for p in range(4):
                if p > 0:
                    kts[p] = kq.tile([128, T], f32r, tag="k", name="kt")
                    qts[p] = kq.tile([128, T], f32r, tag="q", name="qt")
                    kq_proj(kts[p], wkt, p)
                    kq_proj(qts[p], wqt, p)
                kt = kts[p]
                qt_ = qts[p]
                hA, hB = 2 * p, 2 * p + 1
                for qn in range(NT):
                    if p == 0:
                        kq_proj_n(kt, wkt, 0, qn)
                        kq_proj_n(qt_, wqt, 0, qn)
                    q0 = 512 * qn
                    nblocks = 4 * qn + 4
                    ypsA = yps.tile([D + 1, 512], f32, tag="yA", name="ypsA")
                    ypsB = yps.tile([D + 1, 512], f32, tag="yB", name="ypsB")
                    for c in range(nblocks // 2):
                        qkA = qkps.tile([128, 1024], f32, tag="qkA", name="qkA")
                        qkB = qkps.tile([128, 1024], f32, tag="qkB", name="qkB")
                        exA = expp.tile([128, 1024], f32r, tag="exA", name="exA")
                        exB = expp.tile([128, 1024], f32r, tag="exB", name="exB")
                        ms = [max(0, 2 * c + u - 4 * qn) for u in (0, 1)]
                        for u in (0, 1):
                            j = 2 * c + u
                            mcs = 128 * min(ms[u], 2)  # matmul N >= 256
                            for o, qk in ((0, qkA), (D, qkB)):
                                nc.tensor.matmul(
                                    qk[:, 512 * u + mcs:512 * (u + 1)],
                                    kt[o:o + D, 128 * j:128 * (j + 1)],
                                    qt_[o:o + D, q0 + mcs:q0 + 512],
                                    start=True, stop=True)
                        for qk, ex in ((qkA, exA), (qkB, exB)):
                            if ms[1] == 0:  # both blocks fully valid
                                nc.scalar.activation(out=ex[:, :], in_=qk[:, :],
                                                     func=EXP, scale=0.125)
                            else:
                                for u in (0, 1):
                                    cs = 128 * ms[u]
                                    nc.scalar.activation(
                                        out=ex[:, 512 * u + cs:512 * (u + 1)],
                                        in_=qk[:, 512 * u + cs:512 * (u + 1)],
                                        func=EXP, scale=0.125)
                                if ms[1] == 3:
                                    # PV below reads cols 768:896; not
                                    # written by exp -> zero them
                                    nc.vector.memset(
                                        ex[:, 768:896].bitcast(f32), 0.0)
                                st = 128 * ms[0]
                                src = ex[:, st:st + 128]
                                ap3 = bass.AP(
                                    tensor=src.tensor, offset=src.offset,
                                    ap=[src.ap[0], [640, 2], [1, 128]])
                                tri3 = tri[:, :].rearrange("p (a b) -> p a b", a=2)
                                nc.vector.tensor_mul(ap3.bitcast(f32r),
                                                     ap3.bitcast(f32),
                                                     tri3)
                        if p == 0 and c == 0:
                            v_proj_group(qn)
                        for u in (0, 1):
                            j = 2 * c + u
                            mcs = 128 * min(ms[u], 2)
                            for h, yps_t, ex in ((hA, ypsA, exA), (hB, ypsB, exB)):
                                nc.tensor.matmul(
                                    yps_t[0:D + 1, mcs:512],
                                    vsb[j][:, 65 * h:65 * h + 65],
                                    ex[:, 512 * u + mcs:512 * (u + 1)],
                                    start=(j == 0), stop=(j == nblocks - 1))
                    lsq = lr.tile([16, 64], f32, tag="lsq", name="lsq")
                    for h, yps_t in ((hA, ypsA), (hB, ypsB)):
                        o = D * (h % 2)
                        stt = stg.tile([D + 1, 512], f32r, tag="st", name="stt")
                        nc.vector.tensor_copy(stt[:, :], yps_t[0:D + 1, :])
                        nc.sync.dma_start(
                            out=lsq[8 * (h % 2):8 * (h % 2) + 8, :],
                            in_=stt[D:D + 1, :].bitcast(f32))
                        nc.sync.dma_start(
                            out=yts[p][o:o + D, q0:q0 + 512],
                            in_=stt[0:D, :])
                    denom_pipeline(p, qn, lsq)

            # ---- output projection
            for tt in range(NB):
                ps = pps.tile([128, 512], f32, tag="proj", name="pproj")
                for ci in range(4):
                    nc.tensor.matmul(ps[:, :],
                                     yts[ci][:, 128 * tt:128 * (tt + 1)],
                                     wpt[ci][:, :],
                                     start=(ci == 0), stop=(ci == 3))
                ot = osb.tile([128, 512], f32, tag="o", name="ot")
                nc.scalar.copy(ot[:, :], ps[:, :])
                nc.sync.dma_start(out=out[128 * tt:128 * (tt + 1), :], in_=ot[:, :])

    nc.compile()
    return nc


_NC = None


def _get_nc():
    global _NC
    if _NC is None:
        _NC = build_nc()
    return _NC


def _round_f32r(a: np.ndarray) -> np.ndarray:
    """Round fp32 to fp32r (11-bit mantissa) with round-to-nearest."""
    a = np.ascontiguousarray(a, dtype=np.float32)
    u = a.view(np.uint32).astype(np.uint64)
    u = (u + 0x800) & 0xFFFFF000
    return u.astype(np.uint32).view(np.float32)


def kernel(**inputs: np.ndarray) -> np.ndarray:
    x = np.asarray(inputs["x"], dtype=np.float32)
    wqT = _round_f32r(np.asarray(inputs["Wq"], dtype=np.float32).T)
    wkT = _round_f32r(np.asarray(inputs["Wk"], dtype=np.float32).T)
    wvT = _round_f32r(np.asarray(inputs["Wv"], dtype=np.float32).T)
    wpT = _round_f32r(np.asarray(inputs["Wp"], dtype=np.float32).T)
    nc = _get_nc()
    in_maps = []
    for b in range(N_CORES):
        in_maps.append({
            "xT": _round_f32r(x[b].T),
            "wqT": wqT, "wkT": wkT, "wvT": wvT, "wpT": wpT,
        })
    res = run_bass_kernel_spmd(nc, in_maps, core_ids=list(range(N_CORES)))
    return np.stack([res.results[b]["out"] for b in range(N_CORES)], axis=0)


if __name__ == "__main__":
    nc = _get_nc()
    from concourse.timeline_sim import TimelineSim
    print("TimelineSim predicted ns:", TimelineSim(nc).simulate())



# revision 4
# speedup vs baseline: 1.0526x; 1.0526x over previous
"""Causal self-attention Trainium2 kernel (v3, bf16).

Problem: B=8, T=2048, C=512, H=8 heads (D=64), fp32 in/out.
  q = x@Wq.T ; k = x@Wk.T ; v = x@Wv.T  (per head)
  att = softmax(mask(q k^T / sqrt(D)))  ; y = att v ; out = y@Wp.T

Sharding: data-parallel over batch B across 8 NeuronCores (one batch
element per core, weights replicated). No collectives.

Design (all matmuls bf16; PSUM f32; rel err ~4e-3):
  - Scores computed transposed: sT[kpos, q] per (head-pair, q-tile,
    k-chunk) into 2-bank PSUM tiles; ScalarE exp IS the PSUM->SBUF
    evacuation (writes bf16 P^T tiles), exact causal trim per chunk.
  - PV transposed-accumulate: out[q(128), 65] with lhsT = P^T chunk
    (stationary) and rhs = ones-augmented V chunk (65 moving cols,
    col 64 = softmax denominator) accumulated over k-chunks. Bursts
    run per (head, q-block) sequentially so each PSUM bank has at
    most ONE open accumulation group at a time (hardware constraint).
  - Denominators land per-q-partition: reciprocal + broadcast
    tensor-mult scale+evac (no DRAM round trip).
  - y transposed back via PE transpose (bf16) for the output
    projection; out-projection / v-projection / next-pair q,k
    projections are interleaved into the attention j-loops to keep
    ScalarE's exp stream fed.
"""

import numpy as np
import ml_dtypes

import concourse.bass as bass
import concourse.bacc as bacc
import concourse.tile as tile
from concourse import mybir
from concourse.bass_utils import run_bass_kernel_spmd
from concourse.masks import make_identity

B, T, C, H = 8, 2048, 512, 8
D = C // H          # 64
NT = T // 512       # 4 q-tiles of 512
NB = T // 128       # 16 k-blocks of 128
f32 = mybir.dt.float32
bf16 = mybir.dt.bfloat16
EXP = mybir.ActivationFunctionType.Exp
N_CORES = 8
BF = ml_dtypes.bfloat16


def build_nc():
    nc = bacc.Bacc(None)
    xT = nc.dram_tensor("xT", [C, T], bf16, kind="ExternalInput")
    wq = nc.dram_tensor("wqT", [C, C], bf16, kind="ExternalInput")
    wk = nc.dram_tensor("wkT", [C, C], bf16, kind="ExternalInput")
    wv = nc.dram_tensor("wvT", [C, C], bf16, kind="ExternalInput")
    wp = nc.dram_tensor("wpT", [C, C], bf16, kind="ExternalInput")
    out = nc.dram_tensor("out", [T, C], f32, kind="ExternalOutput")

    with tile.TileContext(nc) as tc:
        with tc.tile_pool(name="const", bufs=1) as constp, \
             tc.tile_pool(name="xw", bufs=1) as xw, \
             tc.tile_pool(name="kq", bufs=1) as kqp, \
             tc.tile_pool(name="vp", bufs=1) as vpool, \
             tc.tile_pool(name="ptp", bufs=18) as ptp, \
             tc.tile_pool(name="ys", bufs=2) as ypool, \
             tc.tile_pool(name="rc", bufs=2) as rcp, \
             tc.tile_pool(name="yt", bufs=1) as ytp, \
             tc.tile_pool(name="ob", bufs=2) as otp, \
             tc.tile_pool(name="pp", bufs=1, space="PSUM") as psp, \
             tc.tile_pool(name="qk", bufs=2, space="PSUM") as qkp, \
             tc.tile_pool(name="ac", bufs=1, space="PSUM") as acp, \
             tc.tile_pool(name="mi", bufs=1, space="PSUM") as msp:

            # ---- constants
            tri2 = constp.tile([128, 2, 128], bf16, tag="tri", name="tri2")
            nc.gpsimd.memset(tri2[:, :, :], 1.0)
            for half in range(2):
                sl = tri2[:, half, :]
                nc.gpsimd.affine_select(
                    out=sl, in_=sl, pattern=[[1, 128]], base=0,
                    channel_multiplier=-1,
                    compare_op=mybir.AluOpType.is_ge, fill=0.0)
            ident = constp.tile([128, 128], bf16, tag="id", name="ident")
            make_identity(nc, ident)

            # ---- loads (k/q weights + x cols 0:512 first so compute starts early)
            def load_w(dram, name):
                ws = []
                for ci in range(4):
                    t = xw.tile([128, C], bf16, tag=f"{name}{ci}", name=f"{name}{ci}")
                    nc.sync.dma_start(out=t[:, :], in_=dram[128 * ci:128 * (ci + 1), :])
                    ws.append(t)
                return ws

            xt = [xw.tile([128, T], bf16, tag=f"xT{ci}", name=f"xt{ci}")
                  for ci in range(4)]

            def load_x_n(n):
                for ci in range(4):
                    nc.sync.dma_start(
                        out=xt[ci][:, 512 * n:512 * (n + 1)],
                        in_=xT[128 * ci:128 * (ci + 1), 512 * n:512 * (n + 1)])

            wkt = load_w(wk, "wk")
            load_x_n(0)
            wqt = load_w(wq, "wq")
            wvt = load_w(wv, "wv")
            load_x_n(1)
            load_x_n(2)
            load_x_n(3)
            wpt = load_w(wp, "wp")

            kt = [kqp.tile([128, T], bf16, tag=f"kt{p}", name=f"kt{p}") for p in range(4)]
            qt = [kqp.tile([128, T], bf16, tag=f"qt{p}", name=f"qt{p}") for p in range(4)]
            ytT = [ytp.tile([128, T], bf16, tag=f"yT{p}", name=f"yT{p}") for p in range(4)]
            vsb = [None] * NB

            def kq_proj_n(dst, wt, p, n):
                ps = psp.tile([128, 512], f32, tag="proj", name="pproj")
                for ci in range(4):
                    nc.tensor.matmul(
                        ps[:, :],
                        wt[ci][:, 128 * p:128 * (p + 1)],
                        xt[ci][:, 512 * n:512 * (n + 1)],
                        start=(ci == 0), stop=(ci == 3))
                nc.vector.tensor_copy(dst[:, 512 * n:512 * (n + 1)], ps[:, :])

            def v_proj_tile(tt):
                ps = psp.tile([128, 512], f32, tag="proj", name="pproj")
                for ci in range(4):
                    nc.tensor.matmul(
                        ps[:, :],
                        xt[ci][:, 128 * tt:128 * (tt + 1)],
                        wvt[ci][:, :],
                        start=(ci == 0), stop=(ci == 3))
                # ones-augmented V: [128, 8 heads, 65], col 64 stays 1.0
                vt = vpool.tile([128, 8, 65], bf16, tag=f"v{tt}", name=f"v{tt}")
                nc.gpsimd.memset(vt[:, :, :], 1.0)
                nc.vector.tensor_copy(
                    vt[:, :, 0:64],
                    ps[:, :].rearrange("x (h d) -> x h d", h=8))
                vsb[tt] = vt

            def out_proj_tile(tt):
                ps = psp.tile([128, 512], f32, tag="proj", name="pproj")
                for ci in range(4):
                    nc.tensor.matmul(
                        ps[:, :],
                        ytT[ci][:, 128 * tt:128 * (tt + 1)],
                        wpt[ci][:, :],
                        start=(ci == 0), stop=(ci == 3))
                ot = otp.tile([128, 512], f32, tag="ot", name="ot")
                nc.vector.tensor_copy(ot[:, :], ps[:, :])
                nc.sync.dma_start(out=out[128 * tt:128 * (tt + 1), :], in_=ot[:, :])

            def attention(p, qn, pending):
                """Head pair p, q-tile qn. pending: deque of proj thunks to
                interleave (one after each k-chunk group)."""
                nb = 4 * qn + 4
                # acc[:, u, 65*qb:65*qb+65]: PV accumulator for head 2p+u,
                # q-block qb (col 64 = denominator). One bank per u; at most
                # one open accumulation group per bank at any time.
                acc = acp.tile([128, 2, 512], f32, tag="acc", name="acc")
                pts = [None] * nb
                for j in range(nb):
                    ms_ = max(0, j - 4 * qn)
                    trim = 128 * ms_
                    qk = qkp.tile([128, 2, 512], f32, tag="qk", name="qk")
                    for u in (0, 1):
                        nc.tensor.matmul(
                            qk[:, u, trim:512],
                            kt[p][64 * u:64 * u + 64, 128 * j:128 * (j + 1)],
                            qt[p][64 * u:64 * u + 64, 512 * qn + trim:512 * qn + 512],
                            start=True, stop=True)
                    pt = ptp.tile([128, 2, 512], bf16, tag="pt", name="pt")
                    nc.scalar.activation(out=pt[:, :, trim:512], in_=qk[:, :, trim:512],
                                         func=EXP, scale=0.125)
                    if j >= 4 * qn:
                        sl = pt[:, :, trim:trim + 128]
                        nc.vector.tensor_mul(sl, sl, tri2[:, :, :])
                    pts[j] = pt
                    # PV bursts for q-blocks whose last k-chunk is j
                    if j >= 4 * qn:
                        qb = j - 4 * qn
                        for u in (0, 1):
                            h = 2 * p + u
                            for j2 in range(0, j + 1):
                                nc.tensor.matmul(
                                    acc[:, u, 65 * qb:65 * qb + 65],
                                    pts[j2][:, u, 128 * qb:128 * (qb + 1)],
                                    vsb[j2][:, h, :],
                                    start=(j2 == 0), stop=(j2 == j))
                    if pending:
                        pending.popleft()()
                while pending:
                    pending.popleft()()
                # denominators -> reciprocal -> scale+evac to SBUF bf16
                rec = rcp.tile([128, 2, 4], f32, tag="rec", name="rec")
                den = bass.AP(tensor=acc.tensor, offset=acc.offset + 64,
                              ap=[acc.ap[0], [512, 2], [65, 4], [1, 1]])
                nc.vector.reciprocal(out=rec[:, :, :], in_=den)
                ysb = ypool.tile([128, 4, 2, 64], bf16, tag="ysb", name="ysb")
                acc_r = bass.AP(tensor=acc.tensor, offset=acc.offset,
                                ap=[acc.ap[0], [65, 4], [512, 2], [1, 64]])
                rec_b = bass.AP(tensor=rec.tensor, offset=rec.offset,
                                ap=[rec.ap[0], [1, 4], [4, 2], [0, 64]])
                nc.vector.tensor_mul(ysb[:, :, :, :], acc_r, rec_b)
                # transpose y -> yT (2 heads x 64 = 128 channels per pair)
                mi = msp.tile([128, 256], f32, tag="misc", name="mi")
                tp = mi[:, :].bitcast(bf16).rearrange("x (q c) -> x q c", q=4)
                for qb in range(4):
                    nc.tensor.transpose(tp[:, qb, :], ysb[:, qb, :, :], ident[:, :])
                nc.vector.tensor_copy(
                    ytT[p][:, 512 * qn:512 * (qn + 1)],
                    tp[:, :, :])

            # ---- main pipeline
            from collections import deque
            pend = deque()
            # upfront: v tiles 0..3 and pair-0 projections for qn 0
            for tt in range(4):
                v_proj_tile(tt)
            kq_proj_n(kt[0], wkt, 0, 0)
            kq_proj_n(qt[0], wqt, 0, 0)
            for qn in range(NT):
                for p in range(4):
                    # queue work consumed by LATER attention blocks
                    if p < 3:
                        pend.append(lambda p=p, qn=qn: kq_proj_n(kt[p + 1], wkt, p + 1, qn))
                        pend.append(lambda p=p, qn=qn: kq_proj_n(qt[p + 1], wqt, p + 1, qn))
                    elif qn < 3:
                        pend.append(lambda qn=qn: kq_proj_n(kt[0], wkt, 0, qn + 1))
                        pend.append(lambda qn=qn: kq_proj_n(qt[0], wqt, 0, qn + 1))
                    if p == 1 and qn < 3:
                        for tt in range(4 * qn + 4, 4 * qn + 6):
                            pend.append(lambda tt=tt: v_proj_tile(tt))
                    if p == 2 and qn < 3:
                        for tt in range(4 * qn + 6, 4 * qn + 8):
                            pend.append(lambda tt=tt: v_proj_tile(tt))
                    if p == 0 and qn > 0:
                        for tt in range(4 * (qn - 1), 4 * qn):
                            pend.append(lambda tt=tt: out_proj_tile(tt))
                    attention(p, qn, pend)
            for tt in range(12, 16):
                out_proj_tile(tt)

    nc.compile()
    return nc


_NC = None


def _get_nc():
    global _NC
    if _NC is None:
        _NC = build_nc()
    return _NC


def kernel(**inputs: np.ndarray) -> np.ndarray:
    x = np.asarray(inputs["x"], dtype=np.float32)
    wqT = np.ascontiguousarray(np.asarray(inputs["Wq"], np.float32).T).astype(BF)
    wkT = np.ascontiguousarray(np.asarray(inputs["Wk"], np.float32).T).astype(BF)
    wvT = np.ascontiguousarray(np.asarray(inputs["Wv"], np.float32).T).astype(BF)
    wpT = np.ascontiguousarray(np.asarray(inputs["Wp"], np.float32).T).astype(BF)
    nc = _get_nc()
    in_maps = []
    for b in range(N_CORES):
        in_maps.append({
            "xT": np.ascontiguousarray(x[b].T).astype(BF),
            "wqT": wqT, "wkT": wkT, "wvT": wvT, "wpT": wpT,
        })
    res = run_bass_kernel_spmd(nc, in_maps, core_ids=list(range(N_CORES)))
    return np.stack([res.results[b]["out"] for b in range(N_CORES)], axis=0)


if __name__ == "__main__":
    nc = _get_nc()
    from concourse.timeline_sim import TimelineSim
    print("TimelineSim predicted ns:", TimelineSim(nc).simulate())


# revision 10
# speedup vs baseline: 1.1819x; 1.1228x over previous
"""Causal self-attention Trainium2 kernel (v3, bf16).

Problem: B=8, T=2048, C=512, H=8 heads (D=64), fp32 in/out.
  q = x@Wq.T ; k = x@Wk.T ; v = x@Wv.T  (per head)
  att = softmax(mask(q k^T / sqrt(D)))  ; y = att v ; out = y@Wp.T

Sharding: data-parallel over batch B across 8 NeuronCores (one batch
element per core, weights replicated). No collectives.

Design (all matmuls bf16; PSUM f32; rel err ~4e-3):
  - Scores computed transposed: sT[kpos, q] per (head-pair, q-tile,
    k-chunk) into 2-bank PSUM tiles; ScalarE exp IS the PSUM->SBUF
    evacuation (writes bf16 P^T tiles), exact causal trim per chunk.
  - PV transposed-accumulate: out[q(128), 65] with lhsT = P^T chunk
    (stationary) and rhs = ones-augmented V chunk (65 moving cols,
    col 64 = softmax denominator) accumulated over k-chunks. Bursts
    run per (head, q-block) sequentially so each PSUM bank has at
    most ONE open accumulation group at a time (hardware constraint).
  - Denominators land per-q-partition: reciprocal + broadcast
    tensor-mult scale+evac (no DRAM round trip).
  - y transposed back via PE transpose (bf16) for the output
    projection; out-projection / v-projection / next-pair q,k
    projections are interleaved into the attention j-loops to keep
    ScalarE's exp stream fed.
"""

import numpy as np
import ml_dtypes

import concourse.bass as bass
import concourse.bacc as bacc
import concourse.tile as tile
from concourse import mybir
from concourse.bass_utils import run_bass_kernel_spmd

B, T, C, H = 8, 2048, 512, 8
D = C // H          # 64
NT = T // 512       # 4 q-tiles of 512
NB = T // 128       # 16 k-blocks of 128
f32 = mybir.dt.float32
bf16 = mybir.dt.bfloat16
EXP = mybir.ActivationFunctionType.Exp
N_CORES = 8
BF = ml_dtypes.bfloat16


def build_nc():
    nc = bacc.Bacc(None)
    xT = nc.dram_tensor("xT", [C, T], bf16, kind="ExternalInput")
    wq = nc.dram_tensor("wqT", [C, C], bf16, kind="ExternalInput")
    wk = nc.dram_tensor("wkT", [C, C], bf16, kind="ExternalInput")
    wv = nc.dram_tensor("wvT", [C, C], bf16, kind="ExternalInput")
    wp = nc.dram_tensor("wpT", [C, C], bf16, kind="ExternalInput")
    out = nc.dram_tensor("out", [T, C], f32, kind="ExternalOutput")

    with tile.TileContext(nc) as tc:
        with tc.tile_pool(name="const", bufs=1) as constp, \
             tc.tile_pool(name="xw", bufs=1) as xw, \
             tc.tile_pool(name="kq", bufs=1) as kqp, \
             tc.tile_pool(name="vp", bufs=1) as vpool, \
             tc.tile_pool(name="ptp", bufs=18) as ptp, \
             tc.tile_pool(name="ys", bufs=2) as ypool, \
             tc.tile_pool(name="rc", bufs=2) as rcp, \
             tc.tile_pool(name="yt", bufs=1) as ytp, \
             tc.tile_pool(name="ob", bufs=2) as otp, \
             tc.tile_pool(name="pp", bufs=2, space="PSUM") as psp, \
             tc.tile_pool(name="qk", bufs=2, space="PSUM") as qkp, \
             tc.tile_pool(name="ac", bufs=1, space="PSUM") as acp:

            # ---- constants
            tri2 = constp.tile([128, 2, 128], bf16, tag="tri", name="tri2")
            nc.gpsimd.memset(tri2[:, :, :], 1.0)
            for half in range(2):
                sl = tri2[:, half, :]
                nc.gpsimd.affine_select(
                    out=sl, in_=sl, pattern=[[1, 128]], base=0,
                    channel_multiplier=-1,
                    compare_op=mybir.AluOpType.is_ge, fill=0.0)

            # ---- loads (k/q weights + x cols 0:512 first so compute starts early)
            def load_w(dram, name):
                ws = []
                for ci in range(4):
                    t = xw.tile([128, C], bf16, tag=f"{name}{ci}", name=f"{name}{ci}")
                    nc.sync.dma_start(out=t[:, :], in_=dram[128 * ci:128 * (ci + 1), :])
                    ws.append(t)
                return ws

            xt = [xw.tile([128, T], bf16, tag=f"xT{ci}", name=f"xt{ci}")
                  for ci in range(4)]

            def load_x_n(n):
                for ci in range(4):
                    nc.sync.dma_start(
                        out=xt[ci][:, 512 * n:512 * (n + 1)],
                        in_=xT[128 * ci:128 * (ci + 1), 512 * n:512 * (n + 1)])

            wkt = load_w(wk, "wk")
            load_x_n(0)
            wqt = load_w(wq, "wq")
            wvt = load_w(wv, "wv")
            load_x_n(1)
            load_x_n(2)
            load_x_n(3)
            wpt = load_w(wp, "wp")

            kt = [kqp.tile([128, T], bf16, tag=f"kt{p}", name=f"kt{p}") for p in range(4)]
            qt = [kqp.tile([128, T], bf16, tag=f"qt{p}", name=f"qt{p}") for p in range(4)]
            ytT = [ytp.tile([128, T], bf16, tag=f"yT{p}", name=f"yT{p}") for p in range(4)]
            vsb = [None] * NB

            def kq_proj_n(dst, wt, p, n):
                ps = psp.tile([128, 512], f32, tag="proj", name="pproj")
                for ci in range(4):
                    nc.tensor.matmul(
                        ps[:, :],
                        wt[ci][:, 128 * p:128 * (p + 1)],
                        xt[ci][:, 512 * n:512 * (n + 1)],
                        start=(ci == 0), stop=(ci == 3))
                nc.vector.tensor_copy(dst[:, 512 * n:512 * (n + 1)], ps[:, :])

            def v_proj_tile(tt):
                ps = psp.tile([128, 512], f32, tag="proj", name="pproj")
                for ci in range(4):
                    nc.tensor.matmul(
                        ps[:, :],
                        xt[ci][:, 128 * tt:128 * (tt + 1)],
                        wvt[ci][:, :],
                        start=(ci == 0), stop=(ci == 3))
                # ones-augmented V: [128, 8 heads, 65], col 64 stays 1.0
                vt = vpool.tile([128, 8, 65], bf16, tag=f"v{tt}", name=f"v{tt}")
                nc.gpsimd.memset(vt[:, :, :], 1.0)
                nc.vector.tensor_copy(
                    vt[:, :, 0:64],
                    ps[:, :].rearrange("x (h d) -> x h d", h=8))
                vsb[tt] = vt

            def out_proj_tile(tt):
                ps = psp.tile([128, 512], f32, tag="proj", name="pproj")
                for ci in range(4):
                    nc.tensor.matmul(
                        ps[:, :],
                        ytT[ci][:, 128 * tt:128 * (tt + 1)],
                        wpt[ci][:, :],
                        start=(ci == 0), stop=(ci == 3))
                ot = otp.tile([128, 512], f32, tag="ot", name="ot")
                nc.vector.tensor_copy(ot[:, :], ps[:, :])
                nc.sync.dma_start(out=out[128 * tt:128 * (tt + 1), :], in_=ot[:, :])

            def emit_qk(p, qn, j):
                ms_ = max(0, j - 4 * qn)
                trim = 128 * ms_
                qk = qkp.tile([128, 2, 512], f32, tag="qk", name="qk")
                for u in (0, 1):
                    nc.tensor.matmul(
                        qk[:, u, trim:512],
                        kt[p][64 * u:64 * u + 64, 128 * j:128 * (j + 1)],
                        qt[p][64 * u:64 * u + 64, 512 * qn + trim:512 * qn + 512],
                        start=True, stop=True)
                return qk

            def transpose_y(p, qn, ysb):
                # transpose y -> yT (2 heads x 64 = 128 channels per pair)
                # on the (otherwise idle) DMA XBAR; bf16 supports it.
                for qb in range(4):
                    nc.sync.dma_start_transpose(
                        out=ytT[p][:, 512 * qn + 128 * qb:512 * qn + 128 * (qb + 1)],
                        in_=ysb[:, qb, :, :])

            def attention(p, qn, pending, first_qk, next_pq):
                """Head pair p, q-tile qn. pending: deque of proj thunks to
                interleave (one after each k-chunk group). QK runs one
                iteration ahead of the PV bursts, and the NEXT block's first
                QK is emitted in this block's last iteration, so ScalarE's
                exp stream never waits on the PE."""
                nb = 4 * qn + 4
                # acc[:, u, 65*qb:65*qb+65]: PV accumulator for head 2p+u,
                # q-block qb (col 64 = denominator). One bank per u; at most
                # one open accumulation group per bank at any time.
                acc = acp.tile([128, 2, 512], f32, tag="acc", name="acc")
                pts = [None] * nb
                qk = first_qk
                next_qk = None
                for j in range(nb):
                    ms_ = max(0, j - 4 * qn)
                    trim = 128 * ms_
                    pt = ptp.tile([128, 2, 512], bf16, tag="pt", name="pt")
                    nc.scalar.activation(out=pt[:, :, trim:512], in_=qk[:, :, trim:512],
                                         func=EXP, scale=0.125)
                    if j >= 4 * qn:
                        sl = pt[:, :, trim:trim + 128]
                        nc.vector.tensor_mul(sl, sl, tri2[:, :, :])
                    pts[j] = pt
                    if j + 1 < nb:
                        qk = emit_qk(p, qn, j + 1)
                    elif next_pq is not None:
                        next_qk = emit_qk(next_pq[0], next_pq[1], 0)
                    if pending:
                        pending.popleft()()
                    # PV bursts for q-blocks whose last k-chunk is j
                    if j >= 4 * qn:
                        qb = j - 4 * qn
                        for u in (0, 1):
                            h = 2 * p + u
                            for j2 in range(0, j + 1):
                                nc.tensor.matmul(
                                    acc[:, u, 65 * qb:65 * qb + 65],
                                    pts[j2][:, u, 128 * qb:128 * (qb + 1)],
                                    vsb[j2][:, h, :],
                                    start=(j2 == 0), stop=(j2 == j))
                while pending:
                    pending.popleft()()
                # denominators -> reciprocal -> scale+evac to SBUF bf16
                rec = rcp.tile([128, 2, 4], f32, tag="rec", name="rec")
                den = bass.AP(tensor=acc.tensor, offset=acc.offset + 64,
                              ap=[acc.ap[0], [512, 2], [65, 4], [1, 1]])
                nc.vector.reciprocal(out=rec[:, :, :], in_=den)
                ysb = ypool.tile([128, 4, 2, 64], bf16, tag="ysb", name="ysb")
                acc_r = bass.AP(tensor=acc.tensor, offset=acc.offset,
                                ap=[acc.ap[0], [65, 4], [512, 2], [1, 64]])
                rec_b = bass.AP(tensor=rec.tensor, offset=rec.offset,
                                ap=[rec.ap[0], [1, 4], [4, 2], [0, 64]])
                nc.vector.tensor_mul(ysb[:, :, :, :], acc_r, rec_b)
                return ysb, next_qk

            # ---- main pipeline
            from collections import deque
            pend = deque()
            # upfront: pair-0 projections for qn 0, first QK, v tiles 0..3
            kq_proj_n(kt[0], wkt, 0, 0)
            kq_proj_n(qt[0], wqt, 0, 0)
            cur_qk = emit_qk(0, 0, 0)
            for tt in range(4):
                v_proj_tile(tt)
            blocks = [(qn, p) for qn in range(NT) for p in range(4)]
            for bi, (qn, p) in enumerate(blocks):
                # queue work consumed by LATER attention blocks (1 item is
                # drained per k-chunk iteration; rebalanced so Act-heavy
                # late q-tiles absorb more projection work)
                if p < 3:
                    pend.append(lambda p=p, qn=qn: kq_proj_n(kt[p + 1], wkt, p + 1, qn))
                    pend.append(lambda p=p, qn=qn: kq_proj_n(qt[p + 1], wqt, p + 1, qn))
                elif qn < 3:
                    pend.append(lambda qn=qn: kq_proj_n(kt[0], wkt, 0, qn + 1))
                    pend.append(lambda qn=qn: kq_proj_n(qt[0], wqt, 0, qn + 1))
                if qn < 3:
                    pend.append(lambda tt=4 * qn + 4 + p: v_proj_tile(tt))
                if qn > 0:
                    pend.append(lambda tt=4 * (qn - 1) + p: out_proj_tile(tt))
                nxt = None
                if bi + 1 < len(blocks):
                    nq, np_ = blocks[bi + 1]
                    nxt = (np_, nq)
                ysb, cur_qk = attention(p, qn, pend, cur_qk, nxt)
                transpose_y(p, qn, ysb)
            for tt in range(12, 16):
                out_proj_tile(tt)

    nc.compile()
    return nc


_NC = None


def _get_nc():
    global _NC
    if _NC is None:
        _NC = build_nc()
    return _NC


def kernel(**inputs: np.ndarray) -> np.ndarray:
    x = np.asarray(inputs["x"], dtype=np.float32)
    wqT = np.ascontiguousarray(np.asarray(inputs["Wq"], np.float32).T).astype(BF)
    wkT = np.ascontiguousarray(np.asarray(inputs["Wk"], np.float32).T).astype(BF)
    wvT = np.ascontiguousarray(np.asarray(inputs["Wv"], np.float32).T).astype(BF)
    wpT = np.ascontiguousarray(np.asarray(inputs["Wp"], np.float32).T).astype(BF)
    nc = _get_nc()
    in_maps = []
    for b in range(N_CORES):
        in_maps.append({
            "xT": np.ascontiguousarray(x[b].T).astype(BF),
            "wqT": wqT, "wkT": wkT, "wvT": wvT, "wpT": wpT,
        })
    res = run_bass_kernel_spmd(nc, in_maps, core_ids=list(range(N_CORES)))
    return np.stack([res.results[b]["out"] for b in range(N_CORES)], axis=0)


if __name__ == "__main__":
    nc = _get_nc()
    from concourse.timeline_sim import TimelineSim
    print("TimelineSim predicted ns:", TimelineSim(nc).simulate())


# revision 19
# speedup vs baseline: 1.2038x; 1.0185x over previous
"""Causal self-attention Trainium2 kernel (v3, bf16).

Problem: B=8, T=2048, C=512, H=8 heads (D=64), fp32 in/out.
  q = x@Wq.T ; k = x@Wk.T ; v = x@Wv.T  (per head)
  att = softmax(mask(q k^T / sqrt(D)))  ; y = att v ; out = y@Wp.T

Sharding: data-parallel over batch B across 8 NeuronCores (one batch
element per core, weights replicated). No collectives.

Design (all matmuls bf16; PSUM f32; rel err ~4e-3):
  - Scores computed transposed: sT[kpos, q] per (head-pair, q-tile,
    k-chunk) into 2-bank PSUM tiles; ScalarE exp IS the PSUM->SBUF
    evacuation (writes bf16 P^T tiles), exact causal trim per chunk.
  - PV transposed-accumulate: out[q(128), 65] with lhsT = P^T chunk
    (stationary) and rhs = ones-augmented V chunk (65 moving cols,
    col 64 = softmax denominator) accumulated over k-chunks. Bursts
    run per (head, q-block) sequentially so each PSUM bank has at
    most ONE open accumulation group at a time (hardware constraint).
  - Denominators land per-q-partition: reciprocal + broadcast
    tensor-mult scale+evac (no DRAM round trip).
  - y transposed back via PE transpose (bf16) for the output
    projection; out-projection / v-projection / next-pair q,k
    projections are interleaved into the attention j-loops to keep
    ScalarE's exp stream fed.
"""

import numpy as np
import ml_dtypes

import concourse.bass as bass
import concourse.bacc as bacc
import concourse.tile as tile
from concourse import mybir
from concourse.bass_utils import run_bass_kernel_spmd

B, T, C, H = 8, 2048, 512, 8
D = C // H          # 64
NT = T // 512       # 4 q-tiles of 512
NB = T // 128       # 16 k-blocks of 128
f32 = mybir.dt.float32
bf16 = mybir.dt.bfloat16
EXP = mybir.ActivationFunctionType.Exp
N_CORES = 8
BF = ml_dtypes.bfloat16


def build_nc():
    nc = bacc.Bacc(None)
    xT = nc.dram_tensor("xT", [C, T], bf16, kind="ExternalInput")
    wq = nc.dram_tensor("wqT", [C, C], bf16, kind="ExternalInput")
    wk = nc.dram_tensor("wkT", [C, C], bf16, kind="ExternalInput")
    wv = nc.dram_tensor("wvT", [C, C], bf16, kind="ExternalInput")
    wp = nc.dram_tensor("wpT", [C, C], bf16, kind="ExternalInput")
    out = nc.dram_tensor("out", [T, C], f32, kind="ExternalOutput")

    with tile.TileContext(nc) as tc:
        with tc.tile_pool(name="const", bufs=1) as constp, \
             tc.tile_pool(name="xw", bufs=1) as xw, \
             tc.tile_pool(name="kq", bufs=1) as kqp, \
             tc.tile_pool(name="vp", bufs=1) as vpool, \
             tc.tile_pool(name="ptp", bufs=21) as ptp, \
             tc.tile_pool(name="ys", bufs=2) as ypool, \
             tc.tile_pool(name="rc", bufs=2) as rcp, \
             tc.tile_pool(name="yt", bufs=1) as ytp, \
             tc.tile_pool(name="ob", bufs=2) as otp, \
             tc.tile_pool(name="pp", bufs=2, space="PSUM") as psp, \
             tc.tile_pool(name="qk", bufs=2, space="PSUM") as qkp, \
             tc.tile_pool(name="ac", bufs=1, space="PSUM") as acp:

            # ---- constants
            tri2 = constp.tile([128, 2, 128], bf16, tag="tri", name="tri2")
            nc.gpsimd.memset(tri2[:, :, :], 1.0)
            for half in range(2):
                sl = tri2[:, half, :]
                nc.gpsimd.affine_select(
                    out=sl, in_=sl, pattern=[[1, 128]], base=0,
                    channel_multiplier=-1,
                    compare_op=mybir.AluOpType.is_ge, fill=0.0)

            # ---- loads (k/q weights + x cols 0:512 first so compute starts
            # early). One 3D-AP DMA per weight matrix / x column chunk keeps
            # the SP sequencer (565ns per DMA issue) off the critical path.
            def w_tile(name):
                t = xw.tile([128, 4, C], bf16, tag=name, name=name)
                return t, [t[:, ci, :] for ci in range(4)]

            def load_w_bulk(t, dram):
                nc.sync.dma_start(out=t[:, :, :],
                                  in_=dram[:, :].rearrange("(c p) d -> p c d", c=4))

            xtile = xw.tile([128, 4, T], bf16, tag="xT", name="xtile")
            xt = [xtile[:, ci, :] for ci in range(4)]

            def load_x_n(n):
                nc.sync.dma_start(
                    out=xtile[:, :, 512 * n:512 * (n + 1)],
                    in_=xT[:, 512 * n:512 * (n + 1)].rearrange("(c p) t -> p c t", c=4))

            wk_t, wkt = w_tile("wk")
            wq_t, wqt = w_tile("wq")
            wv_t, wvt = w_tile("wv")
            wp_t, wpt = w_tile("wp")
            # wk first, then x cols 0:512 in chunks (so the first projection
            # accumulation matmuls start per-chunk), then the rest in bulk
            # (each DMA costs ~625ns on the serialized HWDGE device).
            for ci in range(4):
                nc.sync.dma_start(out=wk_t[:, ci, :],
                                  in_=wk[128 * ci:128 * (ci + 1), :])
                nc.sync.dma_start(out=xtile[:, ci, 0:512],
                                  in_=xT[128 * ci:128 * (ci + 1), 0:512])
            load_w_bulk(wq_t, wq)
            load_w_bulk(wv_t, wv)
            load_x_n(1)
            load_x_n(2)
            load_x_n(3)
            load_w_bulk(wp_t, wp)

            kt = [kqp.tile([128, T], bf16, tag=f"kt{p}", name=f"kt{p}") for p in range(4)]
            qt = [kqp.tile([128, T], bf16, tag=f"qt{p}", name=f"qt{p}") for p in range(4)]
            ytT = [ytp.tile([128, T], bf16, tag=f"yT{p}", name=f"yT{p}") for p in range(4)]
            vsb = [None] * NB

            def kq_proj_n(dst, wt, p, n, part=None):
                if part is None or part == 0:
                    ps = psp.tile([128, 512], f32, tag="proj", name="pproj")
                    kq_proj_n.ps = ps
                else:
                    ps = kq_proj_n.ps
                cis = range(4) if part is None else (range(2) if part == 0 else range(2, 4))
                for ci in cis:
                    nc.tensor.matmul(
                        ps[:, :],
                        wt[ci][:, 128 * p:128 * (p + 1)],
                        xt[ci][:, 512 * n:512 * (n + 1)],
                        start=(ci == 0), stop=(ci == 3))
                if part is None or part == 1:
                    nc.vector.tensor_copy(dst[:, 512 * n:512 * (n + 1)], ps[:, :])

            def v_proj_tile(tt, part=None):
                if part is None or part == 0:
                    ps = psp.tile([128, 512], f32, tag="proj", name="pproj")
                    v_proj_tile.ps = ps
                else:
                    ps = v_proj_tile.ps
                cis = range(4) if part is None else (range(2) if part == 0 else range(2, 4))
                for ci in cis:
                    nc.tensor.matmul(
                        ps[:, :],
                        xt[ci][:, 128 * tt:128 * (tt + 1)],
                        wvt[ci][:, :],
                        start=(ci == 0), stop=(ci == 3))
                if part is None or part == 1:
                    # ones-augmented V: [128, 8 heads, 65], col 64 stays 1.0
                    vt = vpool.tile([128, 8, 65], bf16, tag=f"v{tt}", name=f"v{tt}")
                    nc.gpsimd.memset(vt[:, :, 64:65], 1.0)
                    nc.vector.tensor_copy(
                        vt[:, :, 0:64],
                        ps[:, :].rearrange("x (h d) -> x h d", h=8))
                    vsb[tt] = vt

            def out_proj_tile(tt, part=None):
                if part is None or part == 0:
                    ps = psp.tile([128, 512], f32, tag="proj", name="pproj")
                    out_proj_tile.ps = ps
                else:
                    ps = out_proj_tile.ps
                cis = range(4) if part is None else (range(2) if part == 0 else range(2, 4))
                for ci in cis:
                    nc.tensor.matmul(
                        ps[:, :],
                        ytT[ci][:, 128 * tt:128 * (tt + 1)],
                        wpt[ci][:, :],
                        start=(ci == 0), stop=(ci == 3))
                if part is None or part == 1:
                    ot = otp.tile([128, 512], f32, tag="ot", name="ot")
                    nc.vector.tensor_copy(ot[:, :], ps[:, :])
                    nc.sync.dma_start(out=out[128 * tt:128 * (tt + 1), :], in_=ot[:, :])

            def emit_qk(p, qn, j):
                ms_ = max(0, j - 4 * qn)
                trim = 128 * ms_
                qk = qkp.tile([128, 2, 512], f32, tag="qk", name="qk")
                for u in (0, 1):
                    nc.tensor.matmul(
                        qk[:, u, trim:512],
                        kt[p][64 * u:64 * u + 64, 128 * j:128 * (j + 1)],
                        qt[p][64 * u:64 * u + 64, 512 * qn + trim:512 * qn + 512],
                        start=True, stop=True)
                return qk

            def transpose_y(p, qn, ysb):
                # transpose y -> yT (2 heads x 64 = 128 channels per pair)
                # on the (otherwise idle) DMA XBAR; bf16 supports it. One
                # instruction for all 4 q-blocks (HWDGE issue is serialized
                # at ~625ns each): out dim 1 folds into the logical
                # transposed partition dim.
                nc.sync.dma_start_transpose(
                    out=ytT[p][:, 512 * qn:512 * (qn + 1)].rearrange(
                        "x (q c) -> x q c", q=4),
                    in_=ysb[:, :, :, :])

            def drain(pending, budget, now):
                """Emit queued proj work: spend the iteration's spare PE
                budget (ns), then keep going while any queued item's
                deadline is due (FIFO order preserves emission deps)."""
                while pending and (budget > 0
                                   or min(dl for _c, dl, _f in pending) <= now):
                    cost, _dl, fn = pending.popleft()
                    fn()
                    budget -= cost
                return budget

            def attention(p, qn, pending, first_qk, next_pq, bi):
                """Head pair p, q-tile qn. pending: deque of
                (cost, deadline, fn) thunks drained against per-iteration PE
                slack. QK runs one iteration ahead of the PV bursts; the
                NEXT block's first QK is emitted in this block's last
                iteration; the final two PV bursts plus the scale/transpose
                epilogue are deferred into the next block's queue (whose
                early iterations are exp-heavy), so neither engine waits at
                block boundaries."""
                nb = 4 * qn + 4
                # acc[:, u, 65*qb:65*qb+65]: PV accumulator for head 2p+u,
                # q-block qb (col 64 = denominator). One bank per u; at most
                # one open accumulation group per bank at any time.
                acc = acp.tile([128, 2, 512], f32, tag="acc", name="acc")
                pts = [None] * nb
                qk = first_qk
                next_qk = None
                carry = 0.0

                def burst(qb):
                    for u in (0, 1):
                        h = 2 * p + u
                        for j2 in range(0, 4 * qn + qb + 1):
                            nc.tensor.matmul(
                                acc[:, u, 65 * qb:65 * qb + 65],
                                pts[j2][:, u, 128 * qb:128 * (qb + 1)],
                                vsb[j2][:, h, :],
                                start=(j2 == 0), stop=(j2 == 4 * qn + qb))

                def epilogue():
                    # denominators -> reciprocal -> scale+evac -> transpose
                    rec = rcp.tile([128, 2, 4], f32, tag="rec", name="rec")
                    den = bass.AP(tensor=acc.tensor, offset=acc.offset + 64,
                                  ap=[acc.ap[0], [512, 2], [65, 4], [1, 1]])
                    nc.vector.reciprocal(out=rec[:, :, :], in_=den)
                    ysb = ypool.tile([128, 4, 2, 64], bf16, tag="ysb", name="ysb")
                    acc_r = bass.AP(tensor=acc.tensor, offset=acc.offset,
                                    ap=[acc.ap[0], [65, 4], [512, 2], [1, 64]])
                    rec_b = bass.AP(tensor=rec.tensor, offset=rec.offset,
                                    ap=[rec.ap[0], [1, 4], [4, 2], [0, 64]])
                    nc.vector.tensor_mul(ysb[:, :, :, :], acc_r, rec_b)
                    transpose_y(p, qn, ysb)

                for j in range(nb):
                    ms_ = max(0, j - 4 * qn)
                    trim = 128 * ms_
                    pt = ptp.tile([128, 2, 512], bf16, tag="pt", name="pt")
                    nc.scalar.activation(out=pt[:, :, trim:512], in_=qk[:, :, trim:512],
                                         func=EXP, scale=0.125)
                    if j >= 4 * qn:
                        sl = pt[:, :, trim:trim + 128]
                        nc.vector.tensor_mul(sl, sl, tri2[:, :, :])
                    pts[j] = pt
                    exp_ns = 2 * (512 - trim) * 0.833 + 185
                    pe_ns = 0.417 * (1024 - trim)          # next QK
                    if j + 1 < nb:
                        qk = emit_qk(p, qn, j + 1)
                    elif next_pq is not None:
                        next_qk = emit_qk(next_pq[0], next_pq[1], 0)
                    qb_d = j - 4 * qn
                    if 0 <= qb_d <= 1:
                        pe_ns += 0.417 * 65 * 2 * (j + 1)  # inline burst
                    carry = min(drain(pending, exp_ns - pe_ns + min(carry, 0.0),
                                      bi + (j + 1) / 100.0), 500.0)
                    if 0 <= qb_d <= 1:
                        burst(qb_d)
                    elif qb_d >= 2:
                        cost = int(0.417 * 65 * 2 * (j + 1))
                        pending.append((cost, bi + 2, lambda qb=qb_d: burst(qb)))
                while pending and min(dl for _c, dl, _f in pending) <= bi + 1:
                    pending.popleft()[2]()
                pending.append((300, bi + 2.0, epilogue))
                return next_qk

            # ---- main pipeline
            from collections import deque
            pend = deque()
            # upfront: pair-0 projections for qn 0, first QK, v tiles 0..3
            kq_proj_n(kt[0], wkt, 0, 0)
            kq_proj_n(qt[0], wqt, 0, 0)
            cur_qk = emit_qk(0, 0, 0)
            for tt in range(4):
                v_proj_tile(tt)
            blocks = [(qn, p) for qn in range(NT) for p in range(4)]
            for bi, (qn, p) in enumerate(blocks):
                # queue work consumed by LATER attention blocks; drained
                # against per-iteration PE slack, flushed at its deadline
                if p < 3:
                    for part in (0, 1):
                        pend.append((430, bi + 1, lambda p=p, qn=qn, part=part: kq_proj_n(kt[p + 1], wkt, p + 1, qn, part)))
                    for part in (0, 1):
                        pend.append((430, bi + 1, lambda p=p, qn=qn, part=part: kq_proj_n(qt[p + 1], wqt, p + 1, qn, part)))
                elif qn < 3:
                    for part in (0, 1):
                        pend.append((430, bi + 1, lambda qn=qn, part=part: kq_proj_n(kt[0], wkt, 0, qn + 1, part)))
                    for part in (0, 1):
                        pend.append((430, bi + 1, lambda qn=qn, part=part: kq_proj_n(qt[0], wqt, 0, qn + 1, part)))
                if p == 0 and qn > 0:
                    for tt in range(4 * qn, 4 * qn + 4):
                        for part in (0, 1):
                            pend.append((430, bi + max(tt - 4 * qn, 1) / 100.0,
                                         lambda tt=tt, part=part: v_proj_tile(tt, part)))
                if qn > 0:
                    for part in (0, 1):
                        pend.append((430, bi + 4.0, lambda tt=4 * (qn - 1) + p, part=part: out_proj_tile(tt, part)))
                nxt = None
                if bi + 1 < len(blocks):
                    nq, np_ = blocks[bi + 1]
                    nxt = (np_, nq)
                cur_qk = attention(p, qn, pend, cur_qk, nxt, bi)
            while pend:
                pend.popleft()[2]()
            for tt in range(12, 16):
                out_proj_tile(tt)

    nc.compile()
    return nc


_NC = None


def _get_nc():
    global _NC
    if _NC is None:
        _NC = build_nc()
    return _NC


def kernel(**inputs: np.ndarray) -> np.ndarray:
    x = np.asarray(inputs["x"], dtype=np.float32)
    wqT = np.ascontiguousarray(np.asarray(inputs["Wq"], np.float32).T).astype(BF)
    wkT = np.ascontiguousarray(np.asarray(inputs["Wk"], np.float32).T).astype(BF)
    wvT = np.ascontiguousarray(np.asarray(inputs["Wv"], np.float32).T).astype(BF)
    wpT = np.ascontiguousarray(np.asarray(inputs["Wp"], np.float32).T).astype(BF)
    nc = _get_nc()
    in_maps = []
    for b in range(N_CORES):
        in_maps.append({
            "xT": np.ascontiguousarray(x[b].T).astype(BF),
            "wqT": wqT, "wkT": wkT, "wvT": wvT, "wpT": wpT,
        })
    res = run_bass_kernel_spmd(nc, in_maps, core_ids=list(range(N_CORES)))
    return np.stack([res.results[b]["out"] for b in range(N_CORES)], axis=0)


if __name__ == "__main__":
    nc = _get_nc()
    from concourse.timeline_sim import TimelineSim
    print("TimelineSim predicted ns:", TimelineSim(nc).simulate())
